# revision 6
# baseline (speedup 1.0000x reference)
"""Trainium2 Bass kernel for nn_Conv_LI (leaky-integrator + 5x5 'same' conv).

Math: with the reference constants, DT*TAU_MEM_INV = 1.0, so the LI cell
collapses to
    vs[t] = i_t,   i_{t+1} = (i_t - 0.2*i_t) + x_t,   i_0 = 0
(an exponential moving accumulation over time), followed by a per-timestep
5x5 cross-correlation with 'same' zero padding.

Distribution: H is sharded across the 8 cores (64 output rows each). Each
core receives its 64 rows plus a 2-row halo on each side (zero-padded at the
global edges), so no inter-core communication is needed.

Per-core pipeline (all 8 cores run the same program, SPMD):
  - x arrives host-side as bf16, time-shifted by one (vs[t] needs x[t-1]),
    zero-padded to [68, 516] spatially, and laid out [h, t, w] so each
    window DMA reads one contiguous 33 KB run per partition.
  - EMA on VectorE in bf16: one scalar_tensor_tensor per timestep:
        vs[s] = (vs[s-1] * 0.8) + x[s]
  - 5x5 conv on TensorE as 5 PSUM-accumulated banded bf16 matmuls
    (contraction over the h-halo partitions; dx shifts are free-dim AP
    offsets). Two timesteps of one pair go to the two column halves of a
    single [128, 512] PSUM bank via tile_position (0,0)/(0,64), so both
    matmuls run concurrently on the PE array.
  - ScalarE copies PSUM -> SBUF with bf16 downcast; output DMA rides the
    scalar HWDGE ring (input rides the sync ring) and the host upcasts.
"""

import numpy as np

T_FULL, H_FULL, W_FULL = 256, 512, 512
N_CORES = 8
HC = H_FULL // N_CORES  # 64 output rows per core
HP = HC + 4             # 68 partition rows incl 2+2 halo
WP = W_FULL + 4         # 516 padded width
TW = 32                 # timesteps per window
PBLK = 4                # psum pairs per eviction block (4 pairs = 8 steps)
DECAY = 0.8

_PROG_CACHE = {}


def _build_program(t_total):
    import concourse.bacc as bacc
    import concourse.mybir as mybir
    import concourse.tile as tile

    f32 = mybir.dt.float32
    bf16 = mybir.dt.bfloat16
    mult = mybir.AluOpType.mult
    add = mybir.AluOpType.add

    assert t_total % (2 * PBLK * TW // TW) == 0 and t_total % TW == 0
    nwin = t_total // TW
    nblk = TW // (2 * PBLK)  # eviction blocks per window

    nc = bacc.Bacc(None, target_bir_lowering=False)
    x = nc.dram_tensor("x", [HP, t_total, WP], bf16, kind="ExternalInput")
    lw_d = nc.dram_tensor("lw", [HP, 5 * HC], bf16, kind="ExternalInput")
    # out layout [s, h, pair, w]: partition line (s,h) writes 4 KB runs
    out = nc.dram_tensor(
        "out", [2, HC, t_total // 2, W_FULL], bf16, kind="ExternalOutput"
    )

    with tile.TileContext(nc) as tc:
        with (
            tc.tile_pool(name="const", bufs=1) as cpool,
            tc.tile_pool(name="xw", bufs=2) as xpool,
            tc.tile_pool(name="vs", bufs=3) as vpool,
            tc.tile_pool(name="ob", bufs=3) as opool,
            tc.tile_pool(name="ps", bufs=2, space="PSUM") as ppool,
        ):
            lw = cpool.tile([HP, 5 * HC], bf16)
            nc.sync.dma_start(out=lw[:HP, :], in_=lw_d[:, :])
            zt = cpool.tile([HP, WP], bf16)
            nc.vector.memset(zt[:HP, :], 0.0)

            prev = None
            for win in range(nwin):
                t0 = win * TW
                xw = xpool.tile([HP, TW * WP], bf16)
                # Split the window load into 4-timestep sub-DMAs: 4 KB
                # descriptors spread across all 16 SDMA engines (a single
                # 33 KB/partition transfer lands on only 4), and the EMA can
                # start as soon as the first slice arrives.
                for q in range(0, TW, 4):
                    nc.sync.dma_start(
                        out=xw[:HP, q * WP : (q + 4) * WP].rearrange(
                            "h (t w) -> h t w", t=4
                        ),
                        in_=x[:, t0 + q : t0 + q + 4, :],
                    )
                vs = vpool.tile([HP, TW * WP], bf16)
                # Wait-absorbing fence: scalar_tensor_tensor's ISA struct only
                # supports a single sync wait, so soak up the DMA-completion
                # and vs-slot-reuse waits on a cheap copy first.
                nc.vector.tensor_copy(out=vs[:HP, 0:4], in_=xw[:HP, 0:4])
                for s in range(TW):
                    cur = vs[:HP, s * WP : (s + 1) * WP]
                    p = zt[:HP, :] if prev is None else prev
                    nc.vector.scalar_tensor_tensor(
                        out=cur,
                        in0=p,
                        scalar=DECAY,
                        in1=xw[:HP, s * WP : (s + 1) * WP],
                        op0=mult,
                        op1=add,
                    )
                    prev = cur
                for pb in range(nblk):
                    pss = [
                        ppool.tile([2 * HC, W_FULL], f32, name=f"ps{i}")
                        for i in range(PBLK)
                    ]
                    for dx in range(5):
                        lwx = lw[:HP, dx * HC : (dx + 1) * HC]
                        for pr in range(PBLK):
                            for s2 in range(2):
                                tl = (pb * PBLK + pr) * 2 + s2
                                nc.tensor.matmul(
                                    pss[pr][s2 * HC : (s2 + 1) * HC, :],
                                    lwx,
                                    vs[:HP, tl * WP + dx : tl * WP + dx + W_FULL],
                                    start=(dx == 0),
                                    stop=(dx == 4),
                                )
                    ob = opool.tile([2 * HC, PBLK * W_FULL], bf16)
                    for pr in range(PBLK):
                        nc.scalar.copy(
                            out=ob[:, pr * W_FULL : (pr + 1) * W_FULL],
                            in_=pss[pr][:, :],
                        )
                    gpb = win * nblk + pb
                    nc.scalar.dma_start(
                        out=out[:, :, gpb * PBLK : (gpb + 1) * PBLK, :].rearrange(
                            "s h p w -> (s h) p w"
                        ),
                        in_=ob[:, :].rearrange("q (p w) -> q p w", p=PBLK),
                    )
    nc.finalize()
    return nc


def _get_program(t_total):
    if t_total not in _PROG_CACHE:
        _PROG_CACHE[t_total] = _build_program(t_total)
    return _PROG_CACHE[t_total]


def _host_prep(x, k, t_total):
    """Build per-core shifted+padded bf16 inputs and banded lhsT matrices."""
    import ml_dtypes

    x = np.asarray(x, dtype=np.float32)
    k = np.asarray(k, dtype=np.float32)
    # time-shift by one (vs[t] = EMA consumes x[t-1]), zero-pad h/w by 2,
    # cast bf16, and transpose to [h, t, w] for contiguous window DMAs
    xs = np.zeros((t_total, H_FULL + 4, W_FULL + 4), ml_dtypes.bfloat16)
    xs[1:, 2 : H_FULL + 2, 2 : W_FULL + 2] = x[: t_total - 1, 0].astype(
        ml_dtypes.bfloat16
    )
    # banded conv matrices: lhsT[p, dx, j] = k[p - j, dx] for p - j in [0, 5)
    lwh = np.zeros((HP, 5, HC), np.float32)
    j = np.arange(HC)
    for dy in range(5):
        for dx in range(5):
            lwh[j + dy, dx, j] = k[dy, dx]
    lwh = np.ascontiguousarray(
        lwh.reshape(HP, 5 * HC).astype(ml_dtypes.bfloat16)
    )
    in_maps = []
    for c in range(N_CORES):
        xc = np.ascontiguousarray(
            xs[:, c * HC : c * HC + HP, :].transpose(1, 0, 2)
        )
        in_maps.append({"x": xc, "lw": lwh})
    return in_maps


def kernel(x, kernel):
    from concourse.bass_utils import run_bass_kernel_spmd

    t_total = x.shape[0]
    in_maps = _host_prep(x, kernel, t_total)
    nc = _get_program(t_total)
    res = run_bass_kernel_spmd(nc, in_maps, list(range(N_CORES)))
    out = np.empty((t_total, 1, H_FULL, W_FULL), np.float32)
    for c in range(N_CORES):
        # o is [s, h, pair, w]; t = 2*pair + s
        o = np.asarray(res.results[c]["out"]).astype(np.float32)
        out[:, 0, c * HC : (c + 1) * HC, :] = o.transpose(2, 0, 1, 3).reshape(
            t_total, HC, W_FULL
        )
    return out


# revision 15
# speedup vs baseline: 1.1187x; 1.1187x over previous
"""Trainium2 Bass kernel for nn_Conv_LI (leaky-integrator + 5x5 'same' conv).

Math: with the reference constants, DT*TAU_MEM_INV = 1.0, so the LI cell
collapses to
    vs[t] = i_t,   i_{t+1} = (i_t - 0.2*i_t) + x_t,   i_0 = 0
(an exponential moving accumulation over time), followed by a per-timestep
5x5 cross-correlation with 'same' zero padding.

Distribution: H is sharded across the 8 cores (64 output rows each). Each
core receives its 64 rows plus a 2-row halo on each side (zero-padded at the
global edges), so no inter-core communication is needed.

Per-core pipeline (all 8 cores run the same program, SPMD):
  - x arrives host-side as bf16, time-shifted by one (vs[t] needs x[t-1]),
    zero-padded to [68, 516] spatially, and laid out [h, t, w] so each
    window DMA reads one contiguous 33 KB run per partition.
  - EMA on VectorE in bf16: one scalar_tensor_tensor per timestep:
        vs[s] = (vs[s-1] * 0.8) + x[s]
  - 5x5 conv on TensorE as 5 PSUM-accumulated banded bf16 matmuls
    (contraction over the h-halo partitions; dx shifts are free-dim AP
    offsets). Two timesteps of one pair go to the two column halves of a
    single [128, 512] PSUM bank via tile_position (0,0)/(0,64), so both
    matmuls run concurrently on the PE array.
  - ScalarE copies PSUM -> SBUF with bf16 downcast; output DMA rides the
    scalar HWDGE ring (input rides the sync ring) and the host upcasts.
"""

import numpy as np

T_FULL, H_FULL, W_FULL = 256, 512, 512
N_CORES = 8
HC = H_FULL // N_CORES  # 64 output rows per core
HP = HC + 4             # 68 partition rows incl 2+2 halo
WP = W_FULL + 4         # 516 padded width
TW = 32                 # timesteps per window
PBLK = 4                # psum pairs per eviction block (4 pairs = 8 steps)
DECAY = 0.8

_PROG_CACHE = {}


def _build_program(t_total):
    import concourse.bacc as bacc
    import concourse.mybir as mybir
    import concourse.tile as tile

    f32 = mybir.dt.float32
    bf16 = mybir.dt.bfloat16

    assert t_total % (2 * PBLK * TW // TW) == 0 and t_total % TW == 0
    nwin = t_total // TW
    nblk = TW // (2 * PBLK)  # eviction blocks per window

    nc = bacc.Bacc(None, target_bir_lowering=False)
    x = nc.dram_tensor("x", [HP, t_total, WP], bf16, kind="ExternalInput")
    lw_d = nc.dram_tensor("lw", [HP, 5 * HC], bf16, kind="ExternalInput")
    # per-pair eviction rescale vectors (see scaled-EMA note below)
    sc_d = nc.dram_tensor("sc", [2 * HC, TW // 2], f32, kind="ExternalInput")
    # out layout [s, h, pair, w]: partition line (s,h) writes 4 KB runs
    out = nc.dram_tensor(
        "out", [2, HC, t_total // 2, W_FULL], bf16, kind="ExternalOutput"
    )

    with tile.TileContext(nc) as tc:
        with (
            tc.tile_pool(name="const", bufs=1) as cpool,
            tc.tile_pool(name="xw", bufs=2) as xpool,
            tc.tile_pool(name="vs", bufs=2) as vpool,
            tc.tile_pool(name="cw", bufs=2) as wpool,
            tc.tile_pool(name="ob", bufs=3) as opool,
            tc.tile_pool(name="ps", bufs=2, space="PSUM") as ppool,
        ):
            lw = cpool.tile([HP, 5 * HC], bf16)
            nc.sync.dma_start(out=lw[:HP, :], in_=lw_d[:, :])
            sc = cpool.tile([2 * HC, TW // 2], f32)
            nc.sync.dma_start(out=sc[:, :], in_=sc_d[:, :])
            zt = cpool.tile([HP, WP], bf16)
            nc.vector.memset(zt[:HP, :], 0.0)

            prev = None
            for win in range(nwin):
                t0 = win * TW
                xw = xpool.tile([HP, TW * WP], bf16)
                # Split the window load into 4-timestep sub-DMAs: 4 KB
                # descriptors spread across all 16 SDMA engines (a single
                # 33 KB/partition transfer lands on only 4), and the EMA can
                # start as soon as the first slice arrives.
                for q in range(0, TW, 4):
                    nc.sync.dma_start(
                        out=xw[:HP, q * WP : (q + 4) * WP].rearrange(
                            "h (t w) -> h t w", t=4
                        ),
                        in_=x[:, t0 + q : t0 + q + 4, :],
                    )
                vs = vpool.tile([HP, TW * WP], bf16)
                # Wait-absorbing fence: the DVE ALU ops' ISA structs only
                # support a single sync wait, so soak up the DMA-completion
                # and vs-slot-reuse waits on a cheap copy first.
                nc.vector.tensor_copy(out=vs[:HP, 0:4], in_=xw[:HP, 0:4])
                # Scaled EMA: x arrives pre-multiplied by 1.25^((t%TW)+1), so
                # the recurrence vs = 0.8*vs + x becomes a plain running sum
                # u[s] = u[s-1] + y[s] (tensor_add runs 2x on DVE; the STT
                # form has no fast mode). The 0.8^(s+1) is restored by the
                # per-pair PSUM-eviction scale (conv is linear). Window carry:
                # u[-1] = vs_prev_last = u_prev[TW-1] * 0.8^TW.
                if prev is None:
                    carry = zt[:HP, :]
                else:
                    cw = wpool.tile([HP, WP], bf16)
                    nc.vector.tensor_scalar_mul(cw[:HP, :], prev, DECAY**TW)
                    carry = cw[:HP, :]
                for s in range(TW):
                    cur = vs[:HP, s * WP : (s + 1) * WP]
                    p = carry if s == 0 else vs[:HP, (s - 1) * WP : s * WP]
                    nc.vector.tensor_add(cur, p, xw[:HP, s * WP : (s + 1) * WP])
                prev = vs[:HP, (TW - 1) * WP : TW * WP]
                for pb in range(nblk):
                    pss = [
                        ppool.tile([2 * HC, W_FULL], f32, name=f"ps{i}")
                        for i in range(PBLK)
                    ]
                    for dx in range(5):
                        lwx = lw[:HP, dx * HC : (dx + 1) * HC]
                        for pr in range(PBLK):
                            for s2 in range(2):
                                tl = (pb * PBLK + pr) * 2 + s2
                                nc.tensor.matmul(
                                    pss[pr][s2 * HC : (s2 + 1) * HC, :],
                                    lwx,
                                    vs[:HP, tl * WP + dx : tl * WP + dx + W_FULL],
                                    start=(dx == 0),
                                    stop=(dx == 4),
                                )
                    ob = opool.tile([2 * HC, PBLK * W_FULL], bf16)
                    for pr in range(PBLK):
                        q = pb * PBLK + pr
                        nc.scalar.activation(
                            out=ob[:, pr * W_FULL : (pr + 1) * W_FULL],
                            in_=pss[pr][:, :],
                            func=mybir.ActivationFunctionType.Copy,
                            scale=sc[:, q : q + 1],
                        )
                    gpb = win * nblk + pb
                    nc.scalar.dma_start(
                        out=out[:, :, gpb * PBLK : (gpb + 1) * PBLK, :].rearrange(
                            "s h p w -> (s h) p w"
                        ),
                        in_=ob[:, :].rearrange("q (p w) -> q p w", p=PBLK),
                    )
    nc.finalize()
    return nc


def _get_program(t_total):
    if t_total not in _PROG_CACHE:
        _PROG_CACHE[t_total] = _build_program(t_total)
    return _PROG_CACHE[t_total]


def _host_prep(x, k, t_total):
    """Build per-core shifted+padded bf16 inputs and banded lhsT matrices."""
    import ml_dtypes

    x = np.asarray(x, dtype=np.float32)
    k = np.asarray(k, dtype=np.float32)
    # time-shift by one (vs[t] = EMA consumes x[t-1]), zero-pad h/w by 2,
    # pre-scale by 1.25^((t%TW)+1) for the scaled-EMA running sum, cast
    # bf16, and transpose to [h, t, w] for contiguous window DMAs
    tscale = (1.0 / DECAY) ** ((np.arange(t_total) % TW) + 1).astype(np.float32)
    xs = np.zeros((t_total, H_FULL + 4, W_FULL + 4), ml_dtypes.bfloat16)
    xs[1:, 2 : H_FULL + 2, 2 : W_FULL + 2] = (
        x[: t_total - 1, 0] * tscale[1:, None, None]
    ).astype(ml_dtypes.bfloat16)
    # per-pair eviction rescale: partitions 0..63 hold the even timestep of
    # the pair (u-scale 0.8^(2q+1)), 64..127 the odd one (0.8^(2q+2))
    q = np.arange(TW // 2, dtype=np.float32)
    sc = np.empty((2 * HC, TW // 2), np.float32)
    sc[:HC, :] = DECAY ** (2 * q + 1)[None, :]
    sc[HC:, :] = DECAY ** (2 * q + 2)[None, :]
    # banded conv matrices: lhsT[p, dx, j] = k[p - j, dx] for p - j in [0, 5)
    lwh = np.zeros((HP, 5, HC), np.float32)
    j = np.arange(HC)
    for dy in range(5):
        for dx in range(5):
            lwh[j + dy, dx, j] = k[dy, dx]
    lwh = np.ascontiguousarray(
        lwh.reshape(HP, 5 * HC).astype(ml_dtypes.bfloat16)
    )
    in_maps = []
    for c in range(N_CORES):
        xc = np.ascontiguousarray(
            xs[:, c * HC : c * HC + HP, :].transpose(1, 0, 2)
        )
        in_maps.append({"x": xc, "lw": lwh, "sc": sc})
    return in_maps


def kernel(x, kernel):
    from concourse.bass_utils import run_bass_kernel_spmd

    t_total = x.shape[0]
    in_maps = _host_prep(x, kernel, t_total)
    nc = _get_program(t_total)
    res = run_bass_kernel_spmd(nc, in_maps, list(range(N_CORES)))
    out = np.empty((t_total, 1, H_FULL, W_FULL), np.float32)
    for c in range(N_CORES):
        # o is [s, h, pair, w]; t = 2*pair + s
        o = np.asarray(res.results[c]["out"]).astype(np.float32)
        out[:, 0, c * HC : (c + 1) * HC, :] = o.transpose(2, 0, 1, 3).reshape(
            t_total, HC, W_FULL
        )
    return out


# revision 17
# speedup vs baseline: 1.1204x; 1.0015x over previous
"""Trainium2 Bass kernel for nn_Conv_LI (leaky-integrator + 5x5 'same' conv).

Math: with the reference constants, DT*TAU_MEM_INV = 1.0, so the LI cell
collapses to
    vs[t] = i_t,   i_{t+1} = (i_t - 0.2*i_t) + x_t,   i_0 = 0
(an exponential moving accumulation over time), followed by a per-timestep
5x5 cross-correlation with 'same' zero padding.

Distribution: H is sharded across the 8 cores (64 output rows each). Each
core receives its 64 rows plus a 2-row halo on each side (zero-padded at the
global edges), so no inter-core communication is needed.

Per-core pipeline (all 8 cores run the same program, SPMD):
  - x arrives host-side as bf16, time-shifted by one (vs[t] needs x[t-1]),
    zero-padded to [68, 516] spatially, and laid out [h, t, w] so each
    window DMA reads one contiguous 33 KB run per partition.
  - EMA on VectorE in bf16: one scalar_tensor_tensor per timestep:
        vs[s] = (vs[s-1] * 0.8) + x[s]
  - 5x5 conv on TensorE as 5 PSUM-accumulated banded bf16 matmuls
    (contraction over the h-halo partitions; dx shifts are free-dim AP
    offsets). Two timesteps of one pair go to the two column halves of a
    single [128, 512] PSUM bank via tile_position (0,0)/(0,64), so both
    matmuls run concurrently on the PE array.
  - ScalarE copies PSUM -> SBUF with bf16 downcast; output DMA rides the
    scalar HWDGE ring (input rides the sync ring) and the host upcasts.
"""

import numpy as np

T_FULL, H_FULL, W_FULL = 256, 512, 512
N_CORES = 8
HC = H_FULL // N_CORES  # 64 output rows per core
HP = HC + 4             # 68 partition rows incl 2+2 halo
WP = W_FULL + 4         # 516 padded width
TW = 32                 # timesteps per window
PBLK = 4                # psum pairs per eviction block (4 pairs = 8 steps)
DECAY = 0.8

_PROG_CACHE = {}


def _build_program(t_total):
    import concourse.bacc as bacc
    import concourse.mybir as mybir
    import concourse.tile as tile

    f32 = mybir.dt.float32
    bf16 = mybir.dt.bfloat16

    assert t_total % (2 * PBLK * TW // TW) == 0 and t_total % TW == 0
    nwin = t_total // TW
    nblk = TW // (2 * PBLK)  # eviction blocks per window

    nc = bacc.Bacc(None, target_bir_lowering=False)
    x = nc.dram_tensor("x", [HP, t_total, WP], bf16, kind="ExternalInput")
    lw_d = nc.dram_tensor("lw", [HP, 5 * HC], bf16, kind="ExternalInput")
    # per-pair eviction rescale vectors (see scaled-EMA note below)
    sc_d = nc.dram_tensor("sc", [2 * HC, TW // 2], f32, kind="ExternalInput")
    # out layout [s, h, pair, w]: partition line (s,h) writes 4 KB runs
    out = nc.dram_tensor(
        "out", [2, HC, t_total // 2, W_FULL], bf16, kind="ExternalOutput"
    )

    with tile.TileContext(nc) as tc:
        with (
            tc.tile_pool(name="const", bufs=1) as cpool,
            tc.tile_pool(name="xw", bufs=2) as xpool,
            tc.tile_pool(name="vs", bufs=2) as vpool,
            tc.tile_pool(name="cw", bufs=2) as wpool,
            tc.tile_pool(name="ob", bufs=3) as opool,
            tc.tile_pool(name="ps", bufs=2, space="PSUM") as ppool,
        ):
            lw = cpool.tile([HP, 5 * HC], bf16)
            nc.sync.dma_start(out=lw[:HP, :], in_=lw_d[:, :])
            sc = cpool.tile([2 * HC, TW // 2], f32)
            nc.sync.dma_start(out=sc[:, :], in_=sc_d[:, :])
            zt = cpool.tile([HP, WP], bf16)
            nc.vector.memset(zt[:HP, :], 0.0)

            prev = None
            for win in range(nwin):
                t0 = win * TW
                xw = xpool.tile([HP, TW * WP], bf16)
                # Split the window load into 4-timestep sub-DMAs: 4 KB
                # descriptors spread across all 16 SDMA engines (a single
                # 33 KB/partition transfer lands on only 4), and the EMA can
                # start as soon as the first slice arrives.
                for q in range(0, TW, 4):
                    nc.sync.dma_start(
                        out=xw[:HP, q * WP : (q + 4) * WP].rearrange(
                            "h (t w) -> h t w", t=4
                        ),
                        in_=x[:, t0 + q : t0 + q + 4, :],
                    )
                vs = vpool.tile([HP, TW * WP], bf16)
                # Wait-absorbing fence: the DVE ALU ops' ISA structs only
                # support a single sync wait, so soak up the DMA-completion
                # and vs-slot-reuse waits on a cheap copy first.
                nc.vector.tensor_copy(out=vs[:HP, 0:4], in_=xw[:HP, 0:4])
                # Scaled EMA, 2-step unrolled: x arrives host-side as
                # z[s] = xs[s-1]*r^s + xs[s]*r^(s+1) (r = 1/0.8), so the
                # recurrence vs = 0.8*vs + x becomes u[s] = u[s-2] + z[s]
                # (tensor_add runs 2x on DVE; the STT form has no fast mode,
                # and the 2-apart dependency hides the SBUF write-ack that
                # stalls a 1-apart chain). The 0.8^(s+1) is restored by the
                # per-pair PSUM-eviction scale (conv is linear). Window carry:
                # u[-2], u[-1] = u_prev[TW-2], u_prev[TW-1] both * 0.8^TW.
                if prev is None:
                    carry = [zt[:HP, :], zt[:HP, :]]
                else:
                    cw = wpool.tile([HP, 2 * WP], bf16)
                    nc.vector.tensor_scalar_mul(cw[:HP, :], prev, DECAY**TW)
                    carry = [cw[:HP, 0:WP], cw[:HP, WP : 2 * WP]]
                for s in range(TW):
                    cur = vs[:HP, s * WP : (s + 1) * WP]
                    p = carry[s] if s < 2 else vs[:HP, (s - 2) * WP : (s - 1) * WP]
                    nc.vector.tensor_add(cur, p, xw[:HP, s * WP : (s + 1) * WP])
                prev = vs[:HP, (TW - 2) * WP : TW * WP]
                for pb in range(nblk):
                    pss = [
                        ppool.tile([2 * HC, W_FULL], f32, name=f"ps{i}")
                        for i in range(PBLK)
                    ]
                    for dx in range(5):
                        lwx = lw[:HP, dx * HC : (dx + 1) * HC]
                        for pr in range(PBLK):
                            for s2 in range(2):
                                tl = (pb * PBLK + pr) * 2 + s2
                                nc.tensor.matmul(
                                    pss[pr][s2 * HC : (s2 + 1) * HC, :],
                                    lwx,
                                    vs[:HP, tl * WP + dx : tl * WP + dx + W_FULL],
                                    start=(dx == 0),
                                    stop=(dx == 4),
                                )
                    ob = opool.tile([2 * HC, PBLK * W_FULL], bf16)
                    for pr in range(PBLK):
                        q = pb * PBLK + pr
                        nc.scalar.activation(
                            out=ob[:, pr * W_FULL : (pr + 1) * W_FULL],
                            in_=pss[pr][:, :],
                            func=mybir.ActivationFunctionType.Copy,
                            scale=sc[:, q : q + 1],
                        )
                    gpb = win * nblk + pb
                    nc.scalar.dma_start(
                        out=out[:, :, gpb * PBLK : (gpb + 1) * PBLK, :].rearrange(
                            "s h p w -> (s h) p w"
                        ),
                        in_=ob[:, :].rearrange("q (p w) -> q p w", p=PBLK),
                    )
    nc.finalize()
    return nc


def _get_program(t_total):
    if t_total not in _PROG_CACHE:
        _PROG_CACHE[t_total] = _build_program(t_total)
    return _PROG_CACHE[t_total]


def _host_prep(x, k, t_total):
    """Build per-core shifted+padded bf16 inputs and banded lhsT matrices."""
    import ml_dtypes

    x = np.asarray(x, dtype=np.float32)
    k = np.asarray(k, dtype=np.float32)
    # time-shift by one (vs[t] = EMA consumes x[t-1]), zero-pad h/w by 2,
    # pre-scale by 1.25^((t%TW)+1) for the scaled-EMA running sum, cast
    # bf16, and transpose to [h, t, w] for contiguous window DMAs
    # z[t] = xs[t-1]*r^(t%TW) + xs[t]*r^((t%TW)+1), r = 1/0.8, xs[t] = x[t-1]:
    # the 2-step-unrolled running-sum increment u[s] = u[s-2] + z[s]. The
    # 2-apart dependency lets consecutive DVE adds hide the SBUF write-ack
    # latency behind the other parity's op.
    r = 1.0 / DECAY
    s_in_w = (np.arange(t_total) % TW).astype(np.float32)
    xsf = np.zeros((t_total, H_FULL, W_FULL), np.float32)
    xsf[1:] = x[: t_total - 1, 0]
    zf = xsf * (r ** (s_in_w + 1))[:, None, None]
    zf[1:] += xsf[:-1] * (r ** s_in_w[1:])[:, None, None]
    xs = np.zeros((t_total, H_FULL + 4, W_FULL + 4), ml_dtypes.bfloat16)
    xs[:, 2 : H_FULL + 2, 2 : W_FULL + 2] = zf.astype(ml_dtypes.bfloat16)
    del xsf, zf
    # per-pair eviction rescale: partitions 0..63 hold the even timestep of
    # the pair (u-scale 0.8^(2q+1)), 64..127 the odd one (0.8^(2q+2))
    q = np.arange(TW // 2, dtype=np.float32)
    sc = np.empty((2 * HC, TW // 2), np.float32)
    sc[:HC, :] = DECAY ** (2 * q + 1)[None, :]
    sc[HC:, :] = DECAY ** (2 * q + 2)[None, :]
    # banded conv matrices: lhsT[p, dx, j] = k[p - j, dx] for p - j in [0, 5)
    lwh = np.zeros((HP, 5, HC), np.float32)
    j = np.arange(HC)
    for dy in range(5):
        for dx in range(5):
            lwh[j + dy, dx, j] = k[dy, dx]
    lwh = np.ascontiguousarray(
        lwh.reshape(HP, 5 * HC).astype(ml_dtypes.bfloat16)
    )
    in_maps = []
    for c in range(N_CORES):
        xc = np.ascontiguousarray(
            xs[:, c * HC : c * HC + HP, :].transpose(1, 0, 2)
        )
        in_maps.append({"x": xc, "lw": lwh, "sc": sc})
    return in_maps


def kernel(x, kernel):
    from concourse.bass_utils import run_bass_kernel_spmd

    t_total = x.shape[0]
    in_maps = _host_prep(x, kernel, t_total)
    nc = _get_program(t_total)
    res = run_bass_kernel_spmd(nc, in_maps, list(range(N_CORES)))
    out = np.empty((t_total, 1, H_FULL, W_FULL), np.float32)
    for c in range(N_CORES):
        # o is [s, h, pair, w]; t = 2*pair + s
        o = np.asarray(res.results[c]["out"]).astype(np.float32)
        out[:, 0, c * HC : (c + 1) * HC, :] = o.transpose(2, 0, 1, 3).reshape(
            t_total, HC, W_FULL
        )
    return out


# revision 18
# speedup vs baseline: 1.2003x; 1.0713x over previous
"""Trainium2 Bass kernel for nn_Conv_LI (leaky-integrator + 5x5 'same' conv).

Math: with the reference constants, DT*TAU_MEM_INV = 1.0, so the LI cell
collapses to
    vs[t] = i_t,   i_{t+1} = (i_t - 0.2*i_t) + x_t,   i_0 = 0
(an exponential moving accumulation over time), followed by a per-timestep
5x5 cross-correlation with 'same' zero padding.

Distribution: H is sharded across the 8 cores (64 output rows each). Each
core receives its 64 rows plus a 2-row halo on each side (zero-padded at the
global edges), so no inter-core communication is needed.

Per-core pipeline (all 8 cores run the same program, SPMD):
  - x arrives host-side as bf16, time-shifted by one (vs[t] needs x[t-1]),
    zero-padded to [68, 516] spatially, and laid out [h, t, w] so each
    window DMA reads one contiguous 33 KB run per partition.
  - EMA on VectorE in bf16: one scalar_tensor_tensor per timestep:
        vs[s] = (vs[s-1] * 0.8) + x[s]
  - 5x5 conv on TensorE as 5 PSUM-accumulated banded bf16 matmuls
    (contraction over the h-halo partitions; dx shifts are free-dim AP
    offsets). Two timesteps of one pair go to the two column halves of a
    single [128, 512] PSUM bank via tile_position (0,0)/(0,64), so both
    matmuls run concurrently on the PE array.
  - ScalarE copies PSUM -> SBUF with bf16 downcast; output DMA rides the
    scalar HWDGE ring (input rides the sync ring) and the host upcasts.
"""

import numpy as np

T_FULL, H_FULL, W_FULL = 256, 512, 512
N_CORES = 8
HC = H_FULL // N_CORES  # 64 output rows per core
HP = HC + 4             # 68 partition rows incl 2+2 halo
WP = W_FULL + 4         # 516 padded width
TW = 32                 # timesteps per window
PBLK = 4                # psum pairs per eviction block (4 pairs = 8 steps)
DECAY = 0.8

_PROG_CACHE = {}


def _build_program(t_total):
    import concourse.bacc as bacc
    import concourse.mybir as mybir
    import concourse.tile as tile

    f32 = mybir.dt.float32
    bf16 = mybir.dt.bfloat16

    assert t_total % (2 * PBLK * TW // TW) == 0 and t_total % TW == 0
    nwin = t_total // TW
    nblk = TW // (2 * PBLK)  # eviction blocks per window

    nc = bacc.Bacc(None, target_bir_lowering=False)
    x = nc.dram_tensor("x", [HP, t_total, WP], bf16, kind="ExternalInput")
    lw_d = nc.dram_tensor("lw", [HP, 5 * HC], bf16, kind="ExternalInput")
    # per-pair eviction rescale vectors (see scaled-EMA note below)
    sc_d = nc.dram_tensor("sc", [2 * HC, TW // 2], f32, kind="ExternalInput")
    # out layout [s, h, pair, w]: partition line (s,h) writes 4 KB runs
    out = nc.dram_tensor(
        "out", [2, HC, t_total // 2, W_FULL], bf16, kind="ExternalOutput"
    )

    with tile.TileContext(nc) as tc:
        with (
            tc.tile_pool(name="const", bufs=1) as cpool,
            tc.tile_pool(name="xw", bufs=2) as xpool,
            tc.tile_pool(name="vs", bufs=2) as vpool,
            tc.tile_pool(name="cw", bufs=2) as wpool,
            tc.tile_pool(name="ob", bufs=3) as opool,
            tc.tile_pool(name="ps", bufs=2, space="PSUM") as ppool,
        ):
            lw = cpool.tile([HP, 5 * HC], bf16)
            nc.sync.dma_start(out=lw[:HP, :], in_=lw_d[:, :])
            sc = cpool.tile([2 * HC, TW // 2], f32)
            nc.sync.dma_start(out=sc[:, :], in_=sc_d[:, :])
            zt = cpool.tile([HP, WP], bf16)
            nc.vector.memset(zt[:HP, :], 0.0)

            prev = None
            for win in range(nwin):
                t0 = win * TW
                xw = xpool.tile([HP, TW * WP], bf16)
                # Split the window load into 4-timestep sub-DMAs: 4 KB
                # descriptors spread across all 16 SDMA engines (a single
                # 33 KB/partition transfer lands on only 4), and the EMA can
                # start as soon as the first slice arrives.
                for q in range(0, TW, 4):
                    nc.gpsimd.dma_start(
                        out=xw[:HP, q * WP : (q + 4) * WP].rearrange(
                            "h (t w) -> h t w", t=4
                        ),
                        in_=x[:, t0 + q : t0 + q + 4, :],
                    )
                vs = vpool.tile([HP, TW * WP], bf16)
                # Wait-absorbing fence: the DVE ALU ops' ISA structs only
                # support a single sync wait, so soak up the DMA-completion
                # and vs-slot-reuse waits on a cheap copy first.
                nc.vector.tensor_copy(out=vs[:HP, 0:4], in_=xw[:HP, 0:4])
                # Scaled EMA, 2-step unrolled: x arrives host-side as
                # z[s] = xs[s-1]*r^s + xs[s]*r^(s+1) (r = 1/0.8), so the
                # recurrence vs = 0.8*vs + x becomes u[s] = u[s-2] + z[s]
                # (tensor_add runs 2x on DVE; the STT form has no fast mode,
                # and the 2-apart dependency hides the SBUF write-ack that
                # stalls a 1-apart chain). The 0.8^(s+1) is restored by the
                # per-pair PSUM-eviction scale (conv is linear). Window carry:
                # u[-2], u[-1] = u_prev[TW-2], u_prev[TW-1] both * 0.8^TW.
                if prev is None:
                    carry = [zt[:HP, :], zt[:HP, :]]
                else:
                    cw = wpool.tile([HP, 2 * WP], bf16)
                    nc.vector.tensor_scalar_mul(cw[:HP, :], prev, DECAY**TW)
                    carry = [cw[:HP, 0:WP], cw[:HP, WP : 2 * WP]]
                for s in range(TW):
                    cur = vs[:HP, s * WP : (s + 1) * WP]
                    p = carry[s] if s < 2 else vs[:HP, (s - 2) * WP : (s - 1) * WP]
                    nc.vector.tensor_add(cur, p, xw[:HP, s * WP : (s + 1) * WP])
                prev = vs[:HP, (TW - 2) * WP : TW * WP]
                for pb in range(nblk):
                    pss = [
                        ppool.tile([2 * HC, W_FULL], f32, name=f"ps{i}")
                        for i in range(PBLK)
                    ]
                    for dx in range(5):
                        lwx = lw[:HP, dx * HC : (dx + 1) * HC]
                        for pr in range(PBLK):
                            for s2 in range(2):
                                tl = (pb * PBLK + pr) * 2 + s2
                                nc.tensor.matmul(
                                    pss[pr][s2 * HC : (s2 + 1) * HC, :],
                                    lwx,
                                    vs[:HP, tl * WP + dx : tl * WP + dx + W_FULL],
                                    start=(dx == 0),
                                    stop=(dx == 4),
                                )
                    ob = opool.tile([2 * HC, PBLK * W_FULL], bf16)
                    for pr in range(PBLK):
                        q = pb * PBLK + pr
                        nc.scalar.activation(
                            out=ob[:, pr * W_FULL : (pr + 1) * W_FULL],
                            in_=pss[pr][:, :],
                            func=mybir.ActivationFunctionType.Copy,
                            scale=sc[:, q : q + 1],
                        )
                    gpb = win * nblk + pb
                    nc.scalar.dma_start(
                        out=out[:, :, gpb * PBLK : (gpb + 1) * PBLK, :].rearrange(
                            "s h p w -> (s h) p w"
                        ),
                        in_=ob[:, :].rearrange("q (p w) -> q p w", p=PBLK),
                    )
    nc.finalize()
    return nc


def _get_program(t_total):
    if t_total not in _PROG_CACHE:
        _PROG_CACHE[t_total] = _build_program(t_total)
    return _PROG_CACHE[t_total]


def _host_prep(x, k, t_total):
    """Build per-core shifted+padded bf16 inputs and banded lhsT matrices."""
    import ml_dtypes

    x = np.asarray(x, dtype=np.float32)
    k = np.asarray(k, dtype=np.float32)
    # time-shift by one (vs[t] = EMA consumes x[t-1]), zero-pad h/w by 2,
    # pre-scale by 1.25^((t%TW)+1) for the scaled-EMA running sum, cast
    # bf16, and transpose to [h, t, w] for contiguous window DMAs
    # z[t] = xs[t-1]*r^(t%TW) + xs[t]*r^((t%TW)+1), r = 1/0.8, xs[t] = x[t-1]:
    # the 2-step-unrolled running-sum increment u[s] = u[s-2] + z[s]. The
    # 2-apart dependency lets consecutive DVE adds hide the SBUF write-ack
    # latency behind the other parity's op.
    r = 1.0 / DECAY
    s_in_w = (np.arange(t_total) % TW).astype(np.float32)
    xsf = np.zeros((t_total, H_FULL, W_FULL), np.float32)
    xsf[1:] = x[: t_total - 1, 0]
    zf = xsf * (r ** (s_in_w + 1))[:, None, None]
    zf[1:] += xsf[:-1] * (r ** s_in_w[1:])[:, None, None]
    xs = np.zeros((t_total, H_FULL + 4, W_FULL + 4), ml_dtypes.bfloat16)
    xs[:, 2 : H_FULL + 2, 2 : W_FULL + 2] = zf.astype(ml_dtypes.bfloat16)
    del xsf, zf
    # per-pair eviction rescale: partitions 0..63 hold the even timestep of
    # the pair (u-scale 0.8^(2q+1)), 64..127 the odd one (0.8^(2q+2))
    q = np.arange(TW // 2, dtype=np.float32)
    sc = np.empty((2 * HC, TW // 2), np.float32)
    sc[:HC, :] = DECAY ** (2 * q + 1)[None, :]
    sc[HC:, :] = DECAY ** (2 * q + 2)[None, :]
    # banded conv matrices: lhsT[p, dx, j] = k[p - j, dx] for p - j in [0, 5)
    lwh = np.zeros((HP, 5, HC), np.float32)
    j = np.arange(HC)
    for dy in range(5):
        for dx in range(5):
            lwh[j + dy, dx, j] = k[dy, dx]
    lwh = np.ascontiguousarray(
        lwh.reshape(HP, 5 * HC).astype(ml_dtypes.bfloat16)
    )
    in_maps = []
    for c in range(N_CORES):
        xc = np.ascontiguousarray(
            xs[:, c * HC : c * HC + HP, :].transpose(1, 0, 2)
        )
        in_maps.append({"x": xc, "lw": lwh, "sc": sc})
    return in_maps


def kernel(x, kernel):
    from concourse.bass_utils import run_bass_kernel_spmd

    t_total = x.shape[0]
    in_maps = _host_prep(x, kernel, t_total)
    nc = _get_program(t_total)
    res = run_bass_kernel_spmd(nc, in_maps, list(range(N_CORES)))
    out = np.empty((t_total, 1, H_FULL, W_FULL), np.float32)
    for c in range(N_CORES):
        # o is [s, h, pair, w]; t = 2*pair + s
        o = np.asarray(res.results[c]["out"]).astype(np.float32)
        out[:, 0, c * HC : (c + 1) * HC, :] = o.transpose(2, 0, 1, 3).reshape(
            t_total, HC, W_FULL
        )
    return out


# revision 19
# speedup vs baseline: 1.2113x; 1.0092x over previous
"""Trainium2 Bass kernel for nn_Conv_LI (leaky-integrator + 5x5 'same' conv).

Math: with the reference constants, DT*TAU_MEM_INV = 1.0, so the LI cell
collapses to
    vs[t] = i_t,   i_{t+1} = (i_t - 0.2*i_t) + x_t,   i_0 = 0
(an exponential moving accumulation over time), followed by a per-timestep
5x5 cross-correlation with 'same' zero padding.

Distribution: H is sharded across the 8 cores (64 output rows each). Each
core receives its 64 rows plus a 2-row halo on each side (zero-padded at the
global edges), so no inter-core communication is needed.

Per-core pipeline (all 8 cores run the same program, SPMD):
  - x arrives host-side as bf16, time-shifted by one (vs[t] needs x[t-1]),
    zero-padded to [68, 516] spatially, and laid out [h, t, w] so each
    window DMA reads one contiguous 33 KB run per partition.
  - EMA on VectorE in bf16: one scalar_tensor_tensor per timestep:
        vs[s] = (vs[s-1] * 0.8) + x[s]
  - 5x5 conv on TensorE as 5 PSUM-accumulated banded bf16 matmuls
    (contraction over the h-halo partitions; dx shifts are free-dim AP
    offsets). Two timesteps of one pair go to the two column halves of a
    single [128, 512] PSUM bank via tile_position (0,0)/(0,64), so both
    matmuls run concurrently on the PE array.
  - ScalarE copies PSUM -> SBUF with bf16 downcast; output DMA rides the
    scalar HWDGE ring (input rides the sync ring) and the host upcasts.
"""

import numpy as np

T_FULL, H_FULL, W_FULL = 256, 512, 512
N_CORES = 8
HC = H_FULL // N_CORES  # 64 output rows per core
HP = HC + 4             # 68 partition rows incl 2+2 halo
WP = W_FULL + 4         # 516 padded width
TW = 32                 # timesteps per window
PBLK = 4                # psum pairs per eviction block (4 pairs = 8 steps)
DECAY = 0.8

_PROG_CACHE = {}


def _build_program(t_total):
    import concourse.bacc as bacc
    import concourse.mybir as mybir
    import concourse.tile as tile

    f32 = mybir.dt.float32
    bf16 = mybir.dt.bfloat16

    assert t_total % (2 * PBLK * TW // TW) == 0 and t_total % TW == 0
    nwin = t_total // TW
    nblk = TW // (2 * PBLK)  # eviction blocks per window

    nc = bacc.Bacc(None, target_bir_lowering=False)
    x = nc.dram_tensor("x", [HP, t_total, WP], bf16, kind="ExternalInput")
    lw_d = nc.dram_tensor("lw", [HP, 5 * HC], bf16, kind="ExternalInput")
    # per-pair eviction rescale vectors (see scaled-EMA note below)
    sc_d = nc.dram_tensor("sc", [2 * HC, TW // 2], f32, kind="ExternalInput")
    # out layout [s, h, pair, w]: partition line (s,h) writes 4 KB runs
    out = nc.dram_tensor(
        "out", [2, HC, t_total // 2, W_FULL], bf16, kind="ExternalOutput"
    )

    with tile.TileContext(nc) as tc:
        with (
            tc.tile_pool(name="const", bufs=1) as cpool,
            tc.tile_pool(name="xw", bufs=2) as xpool,
            tc.tile_pool(name="vs", bufs=2) as vpool,
            tc.tile_pool(name="cw", bufs=2) as wpool,
            tc.tile_pool(name="ob", bufs=3) as opool,
            tc.tile_pool(name="ps", bufs=2, space="PSUM") as ppool,
        ):
            lw = cpool.tile([HP, 5 * HC], bf16)
            nc.sync.dma_start(out=lw[:HP, :], in_=lw_d[:, :])
            sc = cpool.tile([2 * HC, TW // 2], f32)
            nc.sync.dma_start(out=sc[:, :], in_=sc_d[:, :])
            zt = cpool.tile([HP, WP], bf16)
            nc.vector.memset(zt[:HP, :], 0.0)

            prev = None
            for win in range(nwin):
                t0 = win * TW
                xw = xpool.tile([HP, TW * WP], bf16)
                # Split the window load into 4-timestep sub-DMAs: 4 KB
                # descriptors spread across all 16 SDMA engines (a single
                # 33 KB/partition transfer lands on only 4), and the EMA can
                # start as soon as the first slice arrives.
                # 2-step slices at the head of window 0 let the EMA (and so
                # the first matmuls) start a few us earlier.
                qstep = 2 if win == 0 else 4
                for q in range(0, TW, qstep):
                    nc.gpsimd.dma_start(
                        out=xw[:HP, q * WP : (q + qstep) * WP].rearrange(
                            "h (t w) -> h t w", t=qstep
                        ),
                        in_=x[:, t0 + q : t0 + q + qstep, :],
                    )
                vs = vpool.tile([HP, TW * WP], bf16)
                # Wait-absorbing fence: the DVE ALU ops' ISA structs only
                # support a single sync wait, so soak up the DMA-completion
                # and vs-slot-reuse waits on a cheap copy first.
                nc.vector.tensor_copy(out=vs[:HP, 0:4], in_=xw[:HP, 0:4])
                # Scaled EMA, 2-step unrolled: x arrives host-side as
                # z[s] = xs[s-1]*r^s + xs[s]*r^(s+1) (r = 1/0.8), so the
                # recurrence vs = 0.8*vs + x becomes u[s] = u[s-2] + z[s]
                # (tensor_add runs 2x on DVE; the STT form has no fast mode,
                # and the 2-apart dependency hides the SBUF write-ack that
                # stalls a 1-apart chain). The 0.8^(s+1) is restored by the
                # per-pair PSUM-eviction scale (conv is linear). Window carry:
                # u[-2], u[-1] = u_prev[TW-2], u_prev[TW-1] both * 0.8^TW.
                if prev is None:
                    carry = [zt[:HP, :], zt[:HP, :]]
                else:
                    cw = wpool.tile([HP, 2 * WP], bf16)
                    nc.vector.tensor_scalar_mul(cw[:HP, :], prev, DECAY**TW)
                    carry = [cw[:HP, 0:WP], cw[:HP, WP : 2 * WP]]
                for s in range(TW):
                    cur = vs[:HP, s * WP : (s + 1) * WP]
                    p = carry[s] if s < 2 else vs[:HP, (s - 2) * WP : (s - 1) * WP]
                    nc.vector.tensor_add(cur, p, xw[:HP, s * WP : (s + 1) * WP])
                prev = vs[:HP, (TW - 2) * WP : TW * WP]
                for pb in range(nblk):
                    pss = [
                        ppool.tile([2 * HC, W_FULL], f32, name=f"ps{i}")
                        for i in range(PBLK)
                    ]
                    for dx in range(5):
                        lwx = lw[:HP, dx * HC : (dx + 1) * HC]
                        for pr in range(PBLK):
                            for s2 in range(2):
                                tl = (pb * PBLK + pr) * 2 + s2
                                nc.tensor.matmul(
                                    pss[pr][s2 * HC : (s2 + 1) * HC, :],
                                    lwx,
                                    vs[:HP, tl * WP + dx : tl * WP + dx + W_FULL],
                                    start=(dx == 0),
                                    stop=(dx == 4),
                                )
                    ob = opool.tile([2 * HC, PBLK * W_FULL], bf16)
                    for pr in range(PBLK):
                        q = pb * PBLK + pr
                        nc.scalar.activation(
                            out=ob[:, pr * W_FULL : (pr + 1) * W_FULL],
                            in_=pss[pr][:, :],
                            func=mybir.ActivationFunctionType.Copy,
                            scale=sc[:, q : q + 1],
                        )
                    gpb = win * nblk + pb
                    nc.scalar.dma_start(
                        out=out[:, :, gpb * PBLK : (gpb + 1) * PBLK, :].rearrange(
                            "s h p w -> (s h) p w"
                        ),
                        in_=ob[:, :].rearrange("q (p w) -> q p w", p=PBLK),
                    )
    nc.finalize()
    return nc


def _get_program(t_total):
    if t_total not in _PROG_CACHE:
        _PROG_CACHE[t_total] = _build_program(t_total)
    return _PROG_CACHE[t_total]


def _host_prep(x, k, t_total):
    """Build per-core shifted+padded bf16 inputs and banded lhsT matrices."""
    import ml_dtypes

    x = np.asarray(x, dtype=np.float32)
    k = np.asarray(k, dtype=np.float32)
    # time-shift by one (vs[t] = EMA consumes x[t-1]), zero-pad h/w by 2,
    # pre-scale by 1.25^((t%TW)+1) for the scaled-EMA running sum, cast
    # bf16, and transpose to [h, t, w] for contiguous window DMAs
    # z[t] = xs[t-1]*r^(t%TW) + xs[t]*r^((t%TW)+1), r = 1/0.8, xs[t] = x[t-1]:
    # the 2-step-unrolled running-sum increment u[s] = u[s-2] + z[s]. The
    # 2-apart dependency lets consecutive DVE adds hide the SBUF write-ack
    # latency behind the other parity's op.
    r = 1.0 / DECAY
    s_in_w = (np.arange(t_total) % TW).astype(np.float32)
    xsf = np.zeros((t_total, H_FULL, W_FULL), np.float32)
    xsf[1:] = x[: t_total - 1, 0]
    zf = xsf * (r ** (s_in_w + 1))[:, None, None]
    zf[1:] += xsf[:-1] * (r ** s_in_w[1:])[:, None, None]
    xs = np.zeros((t_total, H_FULL + 4, W_FULL + 4), ml_dtypes.bfloat16)
    xs[:, 2 : H_FULL + 2, 2 : W_FULL + 2] = zf.astype(ml_dtypes.bfloat16)
    del xsf, zf
    # per-pair eviction rescale: partitions 0..63 hold the even timestep of
    # the pair (u-scale 0.8^(2q+1)), 64..127 the odd one (0.8^(2q+2))
    q = np.arange(TW // 2, dtype=np.float32)
    sc = np.empty((2 * HC, TW // 2), np.float32)
    sc[:HC, :] = DECAY ** (2 * q + 1)[None, :]
    sc[HC:, :] = DECAY ** (2 * q + 2)[None, :]
    # banded conv matrices: lhsT[p, dx, j] = k[p - j, dx] for p - j in [0, 5)
    lwh = np.zeros((HP, 5, HC), np.float32)
    j = np.arange(HC)
    for dy in range(5):
        for dx in range(5):
            lwh[j + dy, dx, j] = k[dy, dx]
    lwh = np.ascontiguousarray(
        lwh.reshape(HP, 5 * HC).astype(ml_dtypes.bfloat16)
    )
    in_maps = []
    for c in range(N_CORES):
        xc = np.ascontiguousarray(
            xs[:, c * HC : c * HC + HP, :].transpose(1, 0, 2)
        )
        in_maps.append({"x": xc, "lw": lwh, "sc": sc})
    return in_maps


def kernel(x, kernel):
    from concourse.bass_utils import run_bass_kernel_spmd

    t_total = x.shape[0]
    in_maps = _host_prep(x, kernel, t_total)
    nc = _get_program(t_total)
    res = run_bass_kernel_spmd(nc, in_maps, list(range(N_CORES)))
    out = np.empty((t_total, 1, H_FULL, W_FULL), np.float32)
    for c in range(N_CORES):
        # o is [s, h, pair, w]; t = 2*pair + s
        o = np.asarray(res.results[c]["out"]).astype(np.float32)
        out[:, 0, c * HC : (c + 1) * HC, :] = o.transpose(2, 0, 1, 3).reshape(
            t_total, HC, W_FULL
        )
    return out


# revision 21
# speedup vs baseline: 1.2180x; 1.0056x over previous
"""Trainium2 Bass kernel for nn_Conv_LI (leaky-integrator + 5x5 'same' conv).

Math: with the reference constants, DT*TAU_MEM_INV = 1.0, so the LI cell
collapses to
    vs[t] = i_t,   i_{t+1} = (i_t - 0.2*i_t) + x_t,   i_0 = 0
(an exponential moving accumulation over time), followed by a per-timestep
5x5 cross-correlation with 'same' zero padding.

Distribution: H is sharded across the 8 cores (64 output rows each). Each
core receives its 64 rows plus a 2-row halo on each side (zero-padded at the
global edges), so no inter-core communication is needed.

Per-core pipeline (all 8 cores run the same program, SPMD):
  - x arrives host-side as bf16, time-shifted by one (vs[t] needs x[t-1]),
    zero-padded to [68, 516] spatially, and laid out [h, t, w] so each
    window DMA reads one contiguous 33 KB run per partition.
  - EMA on VectorE in bf16: one scalar_tensor_tensor per timestep:
        vs[s] = (vs[s-1] * 0.8) + x[s]
  - 5x5 conv on TensorE as 5 PSUM-accumulated banded bf16 matmuls
    (contraction over the h-halo partitions; dx shifts are free-dim AP
    offsets). Two timesteps of one pair go to the two column halves of a
    single [128, 512] PSUM bank via tile_position (0,0)/(0,64), so both
    matmuls run concurrently on the PE array.
  - ScalarE copies PSUM -> SBUF with bf16 downcast; output DMA rides the
    scalar HWDGE ring (input rides the sync ring) and the host upcasts.
"""

import numpy as np

T_FULL, H_FULL, W_FULL = 256, 512, 512
N_CORES = 8
HC = H_FULL // N_CORES  # 64 output rows per core
HP = HC + 4             # 68 partition rows incl 2+2 halo
WP = W_FULL + 4         # 516 padded width
TW = 32                 # timesteps per window
PBLK = 4                # psum pairs per eviction block (4 pairs = 8 steps)
DECAY = 0.8

_PROG_CACHE = {}


def _build_program(t_total):
    import concourse.bacc as bacc
    import concourse.mybir as mybir
    import concourse.tile as tile

    f32 = mybir.dt.float32
    bf16 = mybir.dt.bfloat16

    assert t_total % (2 * PBLK * TW // TW) == 0 and t_total % TW == 0
    nwin = t_total // TW
    nblk = TW // (2 * PBLK)  # eviction blocks per window

    nc = bacc.Bacc(None, target_bir_lowering=False)
    x = nc.dram_tensor("x", [HP, t_total, WP], bf16, kind="ExternalInput")
    lw_d = nc.dram_tensor("lw", [HP, 5 * HC], bf16, kind="ExternalInput")
    # per-pair eviction rescale vectors (see scaled-EMA note below)
    sc_d = nc.dram_tensor("sc", [2 * HC, TW // 2], f32, kind="ExternalInput")
    # out layout [s, h, pair, w]: partition line (s,h) writes 4 KB runs
    out = nc.dram_tensor(
        "out", [2, HC, t_total // 2, W_FULL], bf16, kind="ExternalOutput"
    )

    with tile.TileContext(nc) as tc:
        with (
            tc.tile_pool(name="const", bufs=1) as cpool,
            tc.tile_pool(name="xw", bufs=2) as xpool,
            tc.tile_pool(name="vs", bufs=2) as vpool,
            tc.tile_pool(name="cw", bufs=2) as wpool,
            tc.tile_pool(name="ob", bufs=3) as opool,
            tc.tile_pool(name="ps", bufs=2, space="PSUM") as ppool,
        ):
            lw = cpool.tile([HP, 5 * HC], bf16)
            nc.sync.dma_start(out=lw[:HP, :], in_=lw_d[:, :])
            sc = cpool.tile([2 * HC, TW // 2], f32)
            nc.sync.dma_start(out=sc[:, :], in_=sc_d[:, :])
            zt = cpool.tile([HP, WP], bf16)
            nc.vector.memset(zt[:HP, :], 0.0)

            prev = None
            for win in range(nwin):
                t0 = win * TW
                xw = xpool.tile([HP, TW * WP], bf16)
                # Split the window load into 4-timestep sub-DMAs: 4 KB
                # descriptors spread across all 16 SDMA engines (a single
                # 33 KB/partition transfer lands on only 4), and the EMA can
                # start as soon as the first slice arrives.
                # Short slices at the head of window 0 let the EMA (and so
                # the first matmuls) start a few us earlier.
                qsteps = [1, 1, 2] + [4] * ((TW - 4) // 4) if win == 0 else [
                    4
                ] * (TW // 4)
                q = 0
                for qstep in qsteps:
                    nc.gpsimd.dma_start(
                        out=xw[:HP, q * WP : (q + qstep) * WP].rearrange(
                            "h (t w) -> h t w", t=qstep
                        ),
                        in_=x[:, t0 + q : t0 + q + qstep, :],
                    )
                    q += qstep
                vs = vpool.tile([HP, TW * WP], bf16)
                # Wait-absorbing fence: the DVE ALU ops' ISA structs only
                # support a single sync wait, so soak up the DMA-completion
                # and vs-slot-reuse waits on a cheap copy first.
                nc.vector.tensor_copy(out=vs[:HP, 0:4], in_=xw[:HP, 0:4])
                # Scaled EMA, 2-step unrolled: x arrives host-side as
                # z[s] = xs[s-1]*r^s + xs[s]*r^(s+1) (r = 1/0.8), so the
                # recurrence vs = 0.8*vs + x becomes u[s] = u[s-2] + z[s]
                # (tensor_add runs 2x on DVE; the STT form has no fast mode,
                # and the 2-apart dependency hides the SBUF write-ack that
                # stalls a 1-apart chain). The 0.8^(s+1) is restored by the
                # per-pair PSUM-eviction scale (conv is linear). Window carry:
                # u[-2], u[-1] = u_prev[TW-2], u_prev[TW-1] both * 0.8^TW.
                if prev is None:
                    carry = [zt[:HP, :], zt[:HP, :]]
                else:
                    cw = wpool.tile([HP, 2 * WP], bf16)
                    nc.vector.tensor_scalar_mul(cw[:HP, :], prev, DECAY**TW)
                    carry = [cw[:HP, 0:WP], cw[:HP, WP : 2 * WP]]
                for s in range(TW):
                    cur = vs[:HP, s * WP : (s + 1) * WP]
                    p = carry[s] if s < 2 else vs[:HP, (s - 2) * WP : (s - 1) * WP]
                    nc.vector.tensor_add(cur, p, xw[:HP, s * WP : (s + 1) * WP])
                prev = vs[:HP, (TW - 2) * WP : TW * WP]
                for pb in range(nblk):
                    pss = [
                        ppool.tile([2 * HC, W_FULL], f32, name=f"ps{i}")
                        for i in range(PBLK)
                    ]
                    for dx in range(5):
                        lwx = lw[:HP, dx * HC : (dx + 1) * HC]
                        for pr in range(PBLK):
                            for s2 in range(2):
                                tl = (pb * PBLK + pr) * 2 + s2
                                nc.tensor.matmul(
                                    pss[pr][s2 * HC : (s2 + 1) * HC, :],
                                    lwx,
                                    vs[:HP, tl * WP + dx : tl * WP + dx + W_FULL],
                                    start=(dx == 0),
                                    stop=(dx == 4),
                                )
                    ob = opool.tile([2 * HC, PBLK * W_FULL], bf16)
                    for pr in range(PBLK):
                        q = pb * PBLK + pr
                        nc.scalar.activation(
                            out=ob[:, pr * W_FULL : (pr + 1) * W_FULL],
                            in_=pss[pr][:, :],
                            func=mybir.ActivationFunctionType.Copy,
                            scale=sc[:, q : q + 1],
                        )
                    gpb = win * nblk + pb
                    last = win == nwin - 1 and pb == nblk - 1
                    # split the final store so its first half overlaps the
                    # last evictions instead of trailing the kernel
                    for h0, h1 in [(0, 2), (2, PBLK)] if last else [(0, PBLK)]:
                        nc.scalar.dma_start(
                            out=out[
                                :, :, gpb * PBLK + h0 : gpb * PBLK + h1, :
                            ].rearrange("s h p w -> (s h) p w"),
                            in_=ob[:, h0 * W_FULL : h1 * W_FULL].rearrange(
                                "q (p w) -> q p w", p=h1 - h0
                            ),
                        )
    nc.finalize()
    return nc


def _get_program(t_total):
    if t_total not in _PROG_CACHE:
        _PROG_CACHE[t_total] = _build_program(t_total)
    return _PROG_CACHE[t_total]


def _host_prep(x, k, t_total):
    """Build per-core shifted+padded bf16 inputs and banded lhsT matrices."""
    import ml_dtypes

    x = np.asarray(x, dtype=np.float32)
    k = np.asarray(k, dtype=np.float32)
    # time-shift by one (vs[t] = EMA consumes x[t-1]), zero-pad h/w by 2,
    # pre-scale by 1.25^((t%TW)+1) for the scaled-EMA running sum, cast
    # bf16, and transpose to [h, t, w] for contiguous window DMAs
    # z[t] = xs[t-1]*r^(t%TW) + xs[t]*r^((t%TW)+1), r = 1/0.8, xs[t] = x[t-1]:
    # the 2-step-unrolled running-sum increment u[s] = u[s-2] + z[s]. The
    # 2-apart dependency lets consecutive DVE adds hide the SBUF write-ack
    # latency behind the other parity's op.
    r = 1.0 / DECAY
    s_in_w = (np.arange(t_total) % TW).astype(np.float32)
    xsf = np.zeros((t_total, H_FULL, W_FULL), np.float32)
    xsf[1:] = x[: t_total - 1, 0]
    zf = xsf * (r ** (s_in_w + 1))[:, None, None]
    zf[1:] += xsf[:-1] * (r ** s_in_w[1:])[:, None, None]
    xs = np.zeros((t_total, H_FULL + 4, W_FULL + 4), ml_dtypes.bfloat16)
    xs[:, 2 : H_FULL + 2, 2 : W_FULL + 2] = zf.astype(ml_dtypes.bfloat16)
    del xsf, zf
    # per-pair eviction rescale: partitions 0..63 hold the even timestep of
    # the pair (u-scale 0.8^(2q+1)), 64..127 the odd one (0.8^(2q+2))
    q = np.arange(TW // 2, dtype=np.float32)
    sc = np.empty((2 * HC, TW // 2), np.float32)
    sc[:HC, :] = DECAY ** (2 * q + 1)[None, :]
    sc[HC:, :] = DECAY ** (2 * q + 2)[None, :]
    # banded conv matrices: lhsT[p, dx, j] = k[p - j, dx] for p - j in [0, 5)
    lwh = np.zeros((HP, 5, HC), np.float32)
    j = np.arange(HC)
    for dy in range(5):
        for dx in range(5):
            lwh[j + dy, dx, j] = k[dy, dx]
    lwh = np.ascontiguousarray(
        lwh.reshape(HP, 5 * HC).astype(ml_dtypes.bfloat16)
    )
    in_maps = []
    for c in range(N_CORES):
        xc = np.ascontiguousarray(
            xs[:, c * HC : c * HC + HP, :].transpose(1, 0, 2)
        )
        in_maps.append({"x": xc, "lw": lwh, "sc": sc})
    return in_maps


def kernel(x, kernel):
    from concourse.bass_utils import run_bass_kernel_spmd

    t_total = x.shape[0]
    in_maps = _host_prep(x, kernel, t_total)
    nc = _get_program(t_total)
    res = run_bass_kernel_spmd(nc, in_maps, list(range(N_CORES)))
    out = np.empty((t_total, 1, H_FULL, W_FULL), np.float32)
    for c in range(N_CORES):
        # o is [s, h, pair, w]; t = 2*pair + s
        o = np.asarray(res.results[c]["out"]).astype(np.float32)
        out[:, 0, c * HC : (c + 1) * HC, :] = o.transpose(2, 0, 1, 3).reshape(
            t_total, HC, W_FULL
        )
    return out
